# revision 10
# baseline (speedup 1.0000x reference)
"""Contrastive loss kernel for trn2 (8 NeuronCores, SPMD).

Computes (see reference): segment-mean embeddings from f1/csv_ids and
f2/wiki_ids, logits = csv_emb @ wiki_emb.T / T, masked log-softmax losses
along both axes, returns A0*axis0 + A1*axis1.

Strategy v2 (transposed tables + ReduceScatter, host-side final scalars):
  - Host pre-scales f rows by the per-id count reciprocal, sorts each
    core's block by id, pads into windows of 128 ids (variable tiles per
    window, same schedule on every core).
  - Phase A (device): matmul(lhsT=f_tile[128n,128d], rhs=onehot[128n,128c])
    accumulates the [D, C]-layout mean table directly in PSUM; banks of 4
    windows are evacuated as bf16 to a DRAM partial table.
  - bf16 ReduceScatter gives each core its 1024-column strip of each
    table; the csv strip IS the logits lhsT; the wiki strips are
    AllGathered (shared output) into the full [D, W] wiki_T.
  - Logits: per [128, 1024] tile: PE matmul, ACT exp(10x-40) with
    accum_out row-sums; column sums via ones-matmul PSUM accumulation.
  - Pair terms: pairs are redistributed by csv strip on host; per c-window
    one-hot matmuls of dma_gathered wiki rows give M1/M0; DVE dots against
    the (r_c-weighted) csv strip produce u1/u0 partials.
  - Device ships rs_all/colsum/uacc; host does ln + reductions in f64.
"""
import sys
sys.path.insert(0, "/opt/trn_rl_repo")

import numpy as np
import ml_dtypes
from contextlib import ExitStack

import concourse.bass as bass
import concourse.tile as tile
from concourse import bacc, mybir
from concourse.bass_utils import run_bass_kernel_spmd

F32 = mybir.dt.float32
BF16 = mybir.dt.bfloat16
I16 = mybir.dt.int16
I32 = mybir.dt.int32
AF = mybir.ActivationFunctionType
OP = mybir.AluOpType

NCORES = 8
N, D = 131072, 128
C = W = 8192
B = N // NCORES            # rows per core
NWIN = 64                  # id windows of 128 per side
STRIP = C // NCORES        # table columns per core (logits rows)
SWIN = STRIP // 128        # strip windows (phase B / logits subs)
TEMP_INV = 10.0            # 1/temperature
SHIFT = 40.0               # constant log-softmax shift
A0 = A1 = 0.5
PAD_REL = 999.0


# ----------------------------------------------------------------- device ---


def _build(tiles_w, ptw, stop="FULL"):
    """Build the SPMD Bass program.

    tiles_w: tuple of 64 ints, data tiles per id-window (same both sides).
    ptw: pair tiles per strip window (phase B).
    stop: debug knob ("A", "RS", "TAB", "FULL").
    """
    nt = sum(tiles_w)
    ntB = SWIN * ptw
    nc = bacc.Bacc("TRN2", target_bir_lowering=False, debug=False,
                   num_devices=NCORES, num_swdge_queues=4)

    f1b = nc.dram_tensor("f1b", [128, nt * D], BF16, kind="ExternalInput")
    f2b = nc.dram_tensor("f2b", [128, nt * D], BF16, kind="ExternalInput")
    rel_c = nc.dram_tensor("rel_c", [128, nt], F32, kind="ExternalInput")
    rel_w = nc.dram_tensor("rel_w", [128, nt], F32, kind="ExternalInput")
    relB = nc.dram_tensor("relB", [128, ntB], F32, kind="ExternalInput")
    b0B = nc.dram_tensor("b0B", [128, ntB], F32, kind="ExternalInput")
    wgi = nc.dram_tensor("wgi", [128, ntB * 8], I16, kind="ExternalInput")
    rcs = nc.dram_tensor("rcs", [128, STRIP], BF16, kind="ExternalInput")

    rs_out = nc.dram_tensor("rs_out", [128, 64], F32, kind="ExternalOutput")
    cs_out = nc.dram_tensor("cs_out", [1, W], F32, kind="ExternalOutput")
    u_out = nc.dram_tensor("u_out", [128, 2 * SWIN], F32, kind="ExternalOutput")

    with tile.TileContext(nc) as tc, ExitStack() as ctx:
        const = ctx.enter_context(tc.tile_pool(name="const", bufs=1))
        persist = ctx.enter_context(tc.tile_pool(name="persist", bufs=1))
        dram = ctx.enter_context(tc.tile_pool(name="dram", bufs=1, space="DRAM"))

        # ---- constants
        iota_i = const.tile([128, 128], I32)
        nc.gpsimd.iota(iota_i[:], pattern=[[1, 128]], base=0, channel_multiplier=0)
        iota_bf = const.tile([128, 128], BF16)
        nc.vector.tensor_copy(out=iota_bf[:], in_=iota_i[:])
        pid_i = const.tile([128, 1], I32)
        nc.gpsimd.iota(pid_i[:], pattern=[[1, 1]], base=0, channel_multiplier=1)
        pid_f = const.tile([128, 1], F32)
        nc.vector.tensor_copy(out=pid_f[:], in_=pid_i[:])
        ident = const.tile([128, 128], BF16)
        nc.vector.tensor_scalar(out=ident[:], in0=iota_bf[:], scalar1=pid_f[:, 0:1],
                                scalar2=None, op0=OP.is_equal)
        ones_bf = const.tile([128, 1], BF16)
        nc.vector.memset(ones_bf[:], 1.0)
        bias_m40 = const.tile([128, 1], F32)
        nc.vector.memset(bias_m40[:], -SHIFT)

        # ---- small inputs
        rel_t = {}
        for key, src in (("w", rel_w), ("c", rel_c)):
            t = const.tile([128, nt], F32, name=f"rel_{key}_t", tag=f"rel_{key}")
            nc.sync.dma_start(out=t[:], in_=src[:, :])
            rel_t[key] = t
        relB_t = const.tile([128, ntB], F32, name="relB_t", tag="relB")
        nc.sync.dma_start(out=relB_t[:], in_=relB[:, :])
        b0B_t = const.tile([128, ntB], F32, name="b0B_t", tag="b0B")
        nc.sync.dma_start(out=b0B_t[:], in_=b0B[:, :])
        wgi_t = const.tile([128, ntB * 8], I16, name="wgi_t", tag="wgi")
        nc.sync.dma_start(out=wgi_t[:], in_=wgi[:, :])
        rcs_t = const.tile([128, STRIP], BF16, name="rcs_t", tag="rcs")
        nc.sync.dma_start(out=rcs_t[:], in_=rcs[:, :])

        # ---- DRAM scratch
        # bank-contiguous bf16 partial tables: [16 banks, 128 part * 512 col]
        part = {"w": dram.tile([16, 128 * 512], BF16, name="part_w"),
                "c": dram.tile([16, 128 * 512], BF16, name="part_c")}
        strip_d = {"w": dram.tile([1, 128 * STRIP], BF16, name="strip_w"),
                   "c": dram.tile([1, 128 * STRIP], BF16, name="strip_c")}
        wiki_ag = dram.tile([8, 128 * STRIP], BF16, name="wiki_ag",
                            addr_space="Shared")
        wiki_rows = dram.tile([W, D], BF16, name="wiki_rows")

        # window -> (bank, col, start tile, end tile) schedule
        wsched = []
        t0 = 0
        for w, tw in enumerate(tiles_w):
            wsched.append((w // 4, w % 4, t0, t0 + tw))
            t0 += tw

        # ================= phase A =================
        sides = [("w", f2b), ("c", f1b)]
        with tc.tile_pool(name="ga", bufs=2) as gpool, \
             tc.tile_pool(name="wka", bufs=10) as wk, \
             tc.tile_pool(name="evac", bufs=4) as evp, \
             tc.tile_pool(name="psa", bufs=3, space="PSUM") as psa:
            for side, fparam in sides:
                pv = part[side].rearrange("g (p x) -> p g x", p=128)
                fall = gpool.tile([128, nt, D], BF16, tag="fall")
                nc.sync.dma_start(out=fall[:], in_=fparam[:, :])
                grp_ps = None
                for w, (grp, k, ts, te) in enumerate(wsched):
                    for t in range(ts, te):
                        oh = wk.tile([128, 128], BF16, tag="oh")
                        nc.vector.tensor_scalar(
                            out=oh[:], in0=iota_bf[:],
                            scalar1=rel_t[side][:, t:t + 1],
                            scalar2=None, op0=OP.is_equal)
                        if k == 0 and t == ts:
                            grp_ps = psa.tile([128, 512], F32, tag="winps")
                        nc.tensor.matmul(grp_ps[:, k * 128:(k + 1) * 128],
                                         lhsT=fall[:, t, :], rhs=oh[:],
                                         start=(t == ts), stop=(t == te - 1))
                    if k == 3:
                        stg = evp.tile([128, 512], BF16, tag="stg")
                        if grp % 2 == 0:
                            nc.vector.tensor_copy(out=stg[:], in_=grp_ps[:])
                        else:
                            nc.scalar.copy(out=stg[:], in_=grp_ps[:])
                        nc.sync.dma_start(out=pv[:, grp, :], in_=stg[:])
                if stop != "A" and side == "w":
                    # CC order: RS_w, AG_w, RS_c. AG_w is emitted after the
                    # csv compute loop so its sem-wait (on RS_w completion)
                    # does not stall the csv evacuations queued on gpsimd.
                    nc.gpsimd.collective_compute(
                        "ReduceScatter", OP.add,
                        replica_groups=[list(range(NCORES))],
                        ins=[part["w"].rearrange("(s b) x -> s (b x)", b=2).opt()],
                        outs=[strip_d["w"].opt()])
            if stop != "A":
                nc.gpsimd.collective_compute(
                    "AllGather", OP.bypass,
                    replica_groups=[list(range(NCORES))],
                    ins=[strip_d["w"].opt()], outs=[wiki_ag.opt()])
                nc.gpsimd.collective_compute(
                    "ReduceScatter", OP.add,
                    replica_groups=[list(range(NCORES))],
                    ins=[part["c"].rearrange("(s b) x -> s (b x)", b=2).opt()],
                    outs=[strip_d["c"].opt()])

        done = stop in ("A", "RS")
        if done:
            zz = persist.tile([128, 64], F32, tag="zz")
            nc.vector.memset(zz[:], 1.0)
            nc.sync.dma_start(out=rs_out[:, :], in_=zz[:])
            zc = persist.tile([1, W], F32, tag="zc")
            nc.vector.memset(zc[:], 1.0)
            nc.sync.dma_start(out=cs_out[:, :], in_=zc[:])
            zu = persist.tile([128, 2 * SWIN], F32, tag="zu")
            nc.vector.memset(zu[:], 0.0)
            nc.sync.dma_start(out=u_out[:, :], in_=zu[:])

        # ================= tables to SBUF =================
        # wiki_T columns ordered (strip s, bank half b, x) == global id
        wiki_T = persist.tile([128, 8, 2, 512], BF16, tag="wiki_T")
        csv_lhsT = persist.tile([128, STRIP], BF16, tag="csv_lhsT")
        csv_w1 = persist.tile([128, STRIP], BF16, tag="csv_w1")
        if not done:
            agv = wiki_ag.rearrange("s (b p x) -> p s b x", b=2, p=128)
            csv_v = strip_d["c"].rearrange("a (b p x) -> p a b x", b=2, p=128)
            for b in range(2):
                nc.sync.dma_start(out=wiki_T[:, :, b, :], in_=agv[:, :, b, :])
                nc.sync.dma_start(out=csv_lhsT[:, b * 512:(b + 1) * 512],
                                  in_=csv_v[:, 0, b, :])
            nc.vector.tensor_tensor(out=csv_w1[:], in0=csv_lhsT[:],
                                    in1=rcs_t[:], op=OP.mult)

        # wiki_rows: transpose wiki_T tiles -> [W, D] row table in DRAM
        wrv = wiki_rows.rearrange("(t p) d -> p t d", p=128)
        with tc.tile_pool(name="trp", bufs=2, space="PSUM") as pst, \
             tc.tile_pool(name="trs", bufs=2) as trs:
            for g in range(16 if not done else 0):
                wr_stg = trs.tile([128, 4, 128], BF16, tag="wrstg")
                for j in range(4):
                    t = g * 4 + j   # global 128-col block: (s, b, q)
                    s, b, q = t // 8, (t // 4) % 2, t % 4
                    tp = pst.tile([128, 128], BF16, tag="trps")
                    nc.tensor.transpose(
                        tp[:], wiki_T[:, s, b, q * 128:(q + 1) * 128], ident[:])
                    nc.vector.tensor_copy(out=wr_stg[:, j, :], in_=tp[:])
                nc.sync.dma_start(out=wrv[:, g * 4:(g + 1) * 4, :], in_=wr_stg[:])

        if stop == "TAB" and not done:
            done = True
            zz = persist.tile([128, 64], F32, tag="zz")
            nc.vector.memset(zz[:], 1.0)
            nc.sync.dma_start(out=rs_out[:, :], in_=zz[:])
            zc = persist.tile([1, W], F32, tag="zc")
            nc.vector.memset(zc[:], 1.0)
            nc.sync.dma_start(out=cs_out[:, :], in_=zc[:])
            zu = persist.tile([128, 2 * SWIN], F32, tag="zu")
            nc.vector.memset(zu[:], 0.0)
            nc.sync.dma_start(out=u_out[:, :], in_=zu[:])

        # ================= logits + pair terms =================
        rs_all = persist.tile([128, 64], F32, tag="rs_all")
        colsum = persist.tile([1, W], F32, tag="colsum")
        uacc = persist.tile([128, 2 * SWIN], F32, tag="uacc")

        with tc.tile_pool(name="gb", bufs=1) as gb, \
             tc.tile_pool(name="wkl", bufs=3) as wk, \
             tc.tile_pool(name="wkb", bufs=6) as wkb, \
             tc.tile_pool(name="scrp", bufs=2) as scrp, \
             tc.tile_pool(name="psl", bufs=2, space="PSUM") as psl, \
             tc.tile_pool(name="psc", bufs=1, space="PSUM") as psc, \
             tc.tile_pool(name="psm", bufs=2, space="PSUM") as psm:
            # all gathers up-front (4 queues)
            wg = []
            for v in range(SWIN if not done else 0):
                g = gb.tile([128, ptw, D], BF16, name=f"wg{v}", tag=f"wg{v}")
                nc.gpsimd.dma_gather(
                    out_ap=g[:], in_ap=wiki_rows[:, :],
                    idxs_ap=wgi_t[:, v * ptw * 8:(v + 1) * ptw * 8],
                    num_idxs=ptw * 128, num_idxs_reg=ptw * 128, elem_size=D,
                    single_packet=False, queue_num=v % 4)
                wg.append(g)

            for k in range(8 if not done else 0):
                # ---- logits chunk k: 1024 wiki columns
                cs_a = psc.tile([1, 512], F32, tag="cs_a")
                cs_b = psc.tile([1, 512], F32, tag="cs_b")
                for s in range(8):
                    lp = psl.tile([128, 1024], F32, tag="lp")
                    nc.tensor.matmul(
                        lp[:, 0:512], lhsT=csv_lhsT[:, s * 128:(s + 1) * 128],
                        rhs=wiki_T[:, k, 0, :], start=True, stop=True)
                    nc.tensor.matmul(
                        lp[:, 512:1024], lhsT=csv_lhsT[:, s * 128:(s + 1) * 128],
                        rhs=wiki_T[:, k, 1, :], start=True, stop=True)
                    ex = wk.tile([128, 1024], BF16, tag="ex")
                    col = s * 8 + k
                    nc.scalar.activation(
                        out=ex[:], in_=lp[:], func=AF.Exp, scale=TEMP_INV,
                        bias=bias_m40[:, 0:1],
                        accum_out=rs_all[:, col:col + 1])
                    nc.tensor.matmul(cs_a[:], lhsT=ones_bf[:], rhs=ex[:, 0:512],
                                     start=(s == 0), stop=(s == 7))
                    nc.tensor.matmul(cs_b[:], lhsT=ones_bf[:], rhs=ex[:, 512:1024],
                                     start=(s == 0), stop=(s == 7))
                nc.vector.tensor_copy(
                    out=colsum[0:1, k * 1024:k * 1024 + 512], in_=cs_a[:])
                nc.vector.tensor_copy(
                    out=colsum[0:1, k * 1024 + 512:(k + 1) * 1024], in_=cs_b[:])

                # ---- pair window k
                mps = psm.tile([128, 256], F32, tag="mps")
                for j in range(ptw):
                    col = k * ptw + j
                    ohu = wkb.tile([128, 128], BF16, tag="ohu")
                    nc.vector.tensor_scalar(
                        out=ohu[:], in0=iota_bf[:],
                        scalar1=relB_t[:, col:col + 1],
                        scalar2=None, op0=OP.is_equal)
                    oh0 = wkb.tile([128, 128], BF16, tag="oh0")
                    nc.vector.tensor_scalar(
                        out=oh0[:], in0=iota_bf[:],
                        scalar1=relB_t[:, col:col + 1],
                        scalar2=b0B_t[:, col:col + 1],
                        op0=OP.is_equal, op1=OP.mult)
                    nc.tensor.matmul(mps[:, 0:128], lhsT=wg[k][:, j, :],
                                     rhs=ohu[:], start=(j == 0),
                                     stop=(j == ptw - 1))
                    nc.tensor.matmul(mps[:, 128:256], lhsT=wg[k][:, j, :],
                                     rhs=oh0[:], start=(j == 0),
                                     stop=(j == ptw - 1))
                scr = scrp.tile([128, 128], F32, tag="scr")
                nc.vector.scalar_tensor_tensor(
                    out=scr[:], in0=mps[:, 0:128], scalar=1.0, op0=OP.mult,
                    in1=csv_w1[:, k * 128:(k + 1) * 128], op1=OP.mult,
                    accum_out=uacc[:, k:k + 1])
                scr2 = scrp.tile([128, 128], F32, tag="scr2")
                nc.vector.scalar_tensor_tensor(
                    out=scr2[:], in0=mps[:, 128:256], scalar=1.0, op0=OP.mult,
                    in1=csv_lhsT[:, k * 128:(k + 1) * 128], op1=OP.mult,
                    accum_out=uacc[:, SWIN + k:SWIN + k + 1])

        if not done:
            nc.sync.dma_start(out=rs_out[:, :], in_=rs_all[:])
            nc.sync.dma_start(out=cs_out[:, :], in_=colsum[:])
            nc.sync.dma_start(out=u_out[:, :], in_=uacc[:])

    nc.finalize()
    return nc


# ------------------------------------------------------------------- host ---


def _wrap16(a):
    """[num] int16 -> [128, num//16] gather-index layout (16-wrap, 8x repl)."""
    return np.ascontiguousarray(np.tile(a.reshape(-1, 16).T, (8, 1)))


def _col128(a, nt):
    """[nt*128] -> [128, nt] tile-column layout."""
    return np.ascontiguousarray(a.reshape(nt, 128).T)


_CACHE = {}


def _run(inputs, trace=False, tmpdir=None):
    f1 = np.asarray(inputs["f1"], np.float32)
    f2 = np.asarray(inputs["f2"], np.float32)
    ci = np.asarray(inputs["csv_ids"]).astype(np.int64)
    wi = np.asarray(inputs["wiki_ids"]).astype(np.int64)

    cnt_c = np.bincount(ci, minlength=C).astype(np.float64)
    cnt_w = np.bincount(wi, minlength=W).astype(np.float64)
    r_c = (1.0 / np.maximum(cnt_c, 1.0)).astype(np.float32)
    r_w = (1.0 / np.maximum(cnt_w, 1.0)).astype(np.float32)
    g_c = (cnt_c > 0).astype(np.float64)
    g_w = (cnt_w > 0).astype(np.float64)

    # ---- phase A window schedule: per-window tiles from global occupancy,
    # rows of window w split evenly across the 8 cores.
    tiles_w = []
    caps = []   # per window: per-core row capacity
    for w in range(NWIN):
        m = 0
        for ids in (ci, wi):
            gcnt = int(((ids >> 7) == w).sum())
            share = -(-gcnt // NCORES)      # ceil
            m = max(m, -(-share // 128))    # ceil tiles
        tiles_w.append(m)
        caps.append(m * 128)
    tiles_w = tuple(tiles_w)
    nt = sum(tiles_w)

    # ---- phase B pair windows (global, by csv id)
    orderB = np.argsort(ci, kind="stable")
    csB = ci[orderB]
    wsB = wi[orderB]
    b0all = r_w[wsB].astype(np.float32)
    startsB = np.searchsorted(csB, np.arange(NWIN) * 128)
    endsB = np.searchsorted(csB, np.arange(1, NWIN + 1) * 128)
    ptw = max(1, int(max(-(-(endsB - startsB) // 128))))
    ntB = SWIN * ptw

    import os as _os
    stop = _os.environ.get("KSTOP", "FULL")
    key = (tiles_w, ptw, stop)
    if key not in _CACHE:
        _CACHE[key] = _build(tiles_w, ptw, stop=stop)
    nc = _CACHE[key]

    # ---- per-side, per-core phase A data
    def side_prep(f, ids, recip):
        fs = (f * recip[ids][:, None]).astype(ml_dtypes.bfloat16)
        # assign rows: window w of core i gets the i-th even share of the
        # core-local rows with ids in window w
        fb_all, rel_all = [], []
        for i in range(NCORES):
            fb = np.zeros((nt * 128, D), ml_dtypes.bfloat16)
            rel = np.full(nt * 128, PAD_REL, np.float32)
            fb_all.append(fb)
            rel_all.append(rel)
        # global sort by (window, then anything) -> split window rows
        order = np.argsort(ids, kind="stable")
        srt = ids[order]
        ws = np.searchsorted(srt, np.arange(NWIN) * 128)
        we = np.searchsorted(srt, np.arange(1, NWIN + 1) * 128)
        base = 0
        for w in range(NWIN):
            rows = order[ws[w]:we[w]]
            rids = srt[ws[w]:we[w]]
            nw = len(rows)
            share = -(-nw // NCORES)
            for i in range(NCORES):
                sl = slice(i * share, min((i + 1) * share, nw))
                cnt = max(0, sl.stop - sl.start)
                if cnt:
                    fb_all[i][base:base + cnt] = fs[rows[sl]]
                    rel_all[i][base:base + cnt] = (
                        rids[sl] - w * 128).astype(np.float32)
            base += caps[w]
        outs = []
        for i in range(NCORES):
            fbp = np.ascontiguousarray(
                fb_all[i].reshape(nt, 128, D).transpose(1, 0, 2)
                .reshape(128, nt * D))
            outs.append((fbp, _col128(rel_all[i], nt)))
        return outs

    prep_c = side_prep(f1, ci, r_c)
    prep_w = side_prep(f2, wi, r_w)

    # ---- per-core phase B data
    in_maps = []
    for i in range(NCORES):
        relBp = np.full(ntB * 128, PAD_REL, np.float32)
        b0Bp = np.zeros(ntB * 128, np.float32)
        wgp = np.zeros(ntB * 128, np.int16)
        for v in range(SWIN):
            gwv = i * SWIN + v
            s, e = startsB[gwv], endsB[gwv]
            cnt = e - s
            base = v * ptw * 128
            relBp[base:base + cnt] = (csB[s:e] - gwv * 128).astype(np.float32)
            b0Bp[base:base + cnt] = b0all[s:e]
            wgp[base:base + cnt] = wsB[s:e].astype(np.int16)
        wgi_arr = np.concatenate(
            [_wrap16(wgp[v * ptw * 128:(v + 1) * ptw * 128])
             for v in range(SWIN)], axis=1)
        rcs_arr = np.ascontiguousarray(np.broadcast_to(
            r_c[i * STRIP:(i + 1) * STRIP][None, :], (128, STRIP))
        ).astype(ml_dtypes.bfloat16)
        in_maps.append({
            "f1b": prep_c[i][0], "f2b": prep_w[i][0],
            "rel_c": prep_c[i][1], "rel_w": prep_w[i][1],
            "relB": _col128(relBp, ntB), "b0B": _col128(b0Bp, ntB),
            "wgi": wgi_arr, "rcs": rcs_arr,
        })

    res = run_bass_kernel_spmd(nc, in_maps, core_ids=list(range(NCORES)),
                               trace=trace, tmpdir=tmpdir)

    # ---- host combine (f64)
    u1 = u0 = 0.0
    v1 = 0.0
    cs_sum = np.zeros(W, np.float64)
    for i in range(NCORES):
        r = res.results[i]
        ua = np.asarray(r["u_out"], np.float64)
        u1 += ua[:, 0:SWIN].sum()
        u0 += ua[:, SWIN:2 * SWIN].sum()
        rs = np.asarray(r["rs_out"], np.float64)   # [128, 64] col = s*8+k
        rowsum = rs.reshape(128, 8, 8).sum(axis=2)  # [p, s]
        gs = g_c[i * STRIP:(i + 1) * STRIP].reshape(SWIN, 128).T  # [p, s]
        v1 += (np.log(np.maximum(rowsum, 1e-300)) * gs).sum()
        cs_sum += np.asarray(r["cs_out"], np.float64)[0]
    v0 = (np.log(np.maximum(cs_sum, 1e-300)) * g_w).sum()
    G1 = g_c.sum()
    G0 = g_w.sum()
    ax1 = -(TEMP_INV * u1 - (v1 + SHIFT * G1)) / C
    ax0 = -(TEMP_INV * u0 - (v0 + SHIFT * G0)) / W
    loss = A0 * ax0 + A1 * ax1
    return np.float32(loss), res


def kernel(**inputs) -> np.ndarray:
    out, _ = _run(inputs)
    return out
